# revision 23
# baseline (speedup 1.0000x reference)
"""Trainium2 Bass kernel for nn_Encoder_506806141403.

12-layer transformer encoder (D=768, H=12, FF=3072) with the quirk that
attention scores use Q vs V (no K projection) and scale by D**-0.5.

Sharding: 8 cores = 4 batch elements x 2 sequence halves. Each core owns
512 query rows of one batch element. Per layer, each core computes its half's
V projection (token-major, bf16) and the pair exchanges halves via a 2-rank
AllGather, so every core holds all 1024 keys/values of its batch element.

This version runs the projection/FFN matmuls in fp8e4 DoubleRow mode (two
128-deep contraction chunks per instruction, double PE throughput). Weights
are scaled x16 on the host before fp8 quantization; the 1/16 is folded into
the PSUM-eviction ops (scalar_tensor_tensor / ACT scale). Attention scores
stay bf16 (single 128-chunk contraction, DoubleRow not applicable); the
probs@V matmul runs fp8 DoubleRow with the softmax numerators written as fp8
directly by the Exp activation. v_aug carries a 64-wide ones block so the
ctx matmul broadcasts the softmax denominator Z onto PSUM partitions 64-127;
normalization is then a single fused (ctx*8)/Z DVE op per head.

Schedule: per layer V is computed first and the AllGather launched
immediately; Q and all own-half scores overlap the collective; peer-half
scores and ctx pipeline per head-pair once the peer V lands.
"""
import os
import sys

sys.path.insert(0, "/opt/trn_rl_repo")

import numpy as np
import ml_dtypes

import concourse.bass as bass
from concourse.bass import ds
from concourse import bacc
import concourse.tile as tile
from concourse import mybir
from concourse.bass_utils import run_bass_kernel_spmd

P = 128
D = 768
H = 12
DH = 64
FF = 3072
NB_D = 6          # D / P
NB_T = 4          # own tokens 512 / P
NB_K = 8          # full tokens 1024 / P
NB_FF = 24        # FF / P
T_OWN = 512
SCALE = float(D) ** -0.5
LN_EPS = 1e-5
N_LAYERS = int(os.environ.get("KERNEL_N_LAYERS", "12"))

WS = 16.0          # fp8 weight scale
CS = 8.0           # fp8 ctx scale

F32 = mybir.dt.float32
F32R = mybir.dt.float32r
BF16 = mybir.dt.bfloat16
FP8 = mybir.dt.float8e4
AF = mybir.ActivationFunctionType
OP = mybir.AluOpType
PM = mybir.MatmulPerfMode

REPLICA_GROUPS = [[0, 1], [2, 3], [4, 5], [6, 7]]

_cached = {}
_last_results = None


def _register_ntff_hook():
    """Register the axon NTFF profile hook (for trace=True exec timing)."""
    import types
    try:
        import antenv.axon_hooks  # noqa: F401
        return
    except ImportError:
        pass
    try:
        from trn_agent_boot.trn_boot import _ntff_profile_via_ctypes
        import antenv
        hook = _ntff_profile_via_ctypes("/opt/axon/libaxon_pjrt.so")
        mod = types.ModuleType("antenv.axon_hooks")
        mod.get_axon_ntff_profile_hook = lambda: hook
        mod.set_axon_ntff_profile_hook = lambda h: None
        sys.modules["antenv.axon_hooks"] = mod
        antenv.axon_hooks = mod
    except Exception:
        pass


def _regions():
    return ((0, 512), (512, 768))


def build(n_layers=N_LAYERS):
    nc = bacc.Bacc(None, target_bir_lowering=False, num_devices=8)
    L = n_layers

    x_d = nc.dram_tensor("x", [P, NB_T, D], F32R, kind="ExternalInput")
    wq_d = nc.dram_tensor("wq", [L, P, NB_D * D], FP8, kind="ExternalInput")
    wv_d = nc.dram_tensor("wv", [L, P, NB_D * D], FP8, kind="ExternalInput")
    wo_d = nc.dram_tensor("wo", [L, P, NB_D * D], FP8, kind="ExternalInput")
    w1_d = nc.dram_tensor("w1", [L, 4, P, NB_D * D], BF16, kind="ExternalInput")
    w2_d = nc.dram_tensor("w2", [L, 4, P, NB_D * D], BF16, kind="ExternalInput")
    bq_d = nc.dram_tensor("bq", [P, L, NB_D], F32, kind="ExternalInput")
    b1_d = nc.dram_tensor("b1", [P, L, NB_FF], F32, kind="ExternalInput")
    bvr_d = nc.dram_tensor("bv_row", [1, L, D], F32, kind="ExternalInput")
    bor_d = nc.dram_tensor("bo_row", [1, L, D], BF16, kind="ExternalInput")
    b2r_d = nc.dram_tensor("b2_row", [1, L, D], BF16, kind="ExternalInput")
    idbf_d = nc.dram_tensor("identbf", [P, P], BF16, kind="ExternalInput")
    ones_d = nc.dram_tensor("ones1", [1, P], BF16, kind="ExternalInput")
    out_d = nc.dram_tensor("out", [P, NB_T, D], F32R, kind="ExternalOutput")

    DBG = bool(int(os.environ.get("KERNEL_DEBUG", "0")))
    dbg = {}
    if DBG:
        dbg["h1fm"] = nc.dram_tensor("dbg_h1fm", [P, NB_D, T_OWN], FP8, kind="ExternalOutput")
        dbg["vsend"] = nc.dram_tensor("dbg_vsend", [P, NB_T, D], BF16, kind="ExternalOutput")
        dbg["vfm8"] = nc.dram_tensor("dbg_vfm8", [P, NB_D, NB_K * P], FP8, kind="ExternalOutput")
        dbg["vaug"] = nc.dram_tensor("dbg_vaug", [P, NB_K, H, P], FP8, kind="ExternalOutput")
        dbg["qpar0"] = nc.dram_tensor("dbg_qpar0", [P, NB_D, T_OWN], FP8, kind="ExternalOutput")
        dbg["qpar1"] = nc.dram_tensor("dbg_qpar1", [P, NB_D, T_OWN], FP8, kind="ExternalOutput")
        dbg["ex00"] = nc.dram_tensor("dbg_ex00", [P, 2, 512], FP8, kind="ExternalOutput")
        dbg["ex01"] = nc.dram_tensor("dbg_ex01", [P, 2, 512], FP8, kind="ExternalOutput")
        dbg["ctxn"] = nc.dram_tensor("dbg_ctxn", [P, NB_D, T_OWN], FP8, kind="ExternalOutput")
        dbg["skip"] = nc.dram_tensor("dbg_skip", [P, NB_T, D], F32R, kind="ExternalOutput")

    with tile.TileContext(nc) as tc:
        with (
            tc.tile_pool(name="state", bufs=1) as st,
            tc.tile_pool(name="stream", bufs=2) as sp,
            tc.tile_pool(name="acts", bufs=1) as ap,
            tc.tile_pool(name="wpool", bufs=2) as wp,
            tc.tile_pool(name="psA", bufs=2, space="PSUM") as psA,
            tc.tile_pool(name="psB", bufs=4, space="PSUM") as psB,
            tc.tile_pool(name="dram", bufs=2, space="DRAM") as dp,
        ):
            # ---- constants ----
            idbf = st.tile([P, P], BF16)
            ones1 = st.tile([1, P], BF16)
            bq_all = st.tile([P, L, NB_D], F32)
            b1_all = st.tile([P, L, NB_FF], F32)
            eps_t = st.tile([P, 1], F32)
            q_par = [st.tile([P, NB_D, T_OWN], FP8, name=f"qpar{p}") for p in range(2)]
            nc.vector.memset(q_par[0][:], 0.0)
            nc.vector.memset(q_par[1][:], 0.0)
            # v_aug: [128 tokens, kb, head, 128] fp8; cols 0:64 = v features,
            # cols 64:128 = 1.0 so the ctx matmul broadcasts Z on rows 64:127.
            v_aug = st.tile([P, NB_K, H, P], FP8, name="vaug")
            nc.vector.memset(v_aug[:, :, :, 64:128], 1.0)
            nc.sync.dma_start(idbf[:], idbf_d[:])
            nc.sync.dma_start(ones1[:], ones_d[:])
            nc.sync.dma_start(bq_all[:], bq_d[:])
            nc.sync.dma_start(b1_all[:], b1_d[:])
            nc.vector.memset(eps_t[:], LN_EPS)

            def ln(out_ap, in_ap):
                """LayerNorm (normalize only) along free axis of [128, 768]."""
                t = ap.tile([P, 32], F32, tag="lnscratch", bufs=3, name="lnt")
                stt = t[:, 0:18].rearrange("p (g s) -> p g s", s=6)
                xg = in_ap.rearrange("p (g d) -> p g d", g=3)
                for g in range(3):
                    nc.vector.bn_stats(stt[:, g, :], xg[:, g, :])
                mv = t[:, 18:20]
                nc.vector.bn_aggr(mv[:], stt[:])
                nc.scalar.activation(t[:, 20:21], mv[:, 1:2], AF.Ln, bias=eps_t[:], scale=1.0)
                nc.scalar.activation(t[:, 21:22], t[:, 20:21], AF.Exp, scale=-0.5)
                nc.vector.tensor_scalar(
                    out=out_ap, in0=in_ap, scalar1=mv[:, 0:1], scalar2=t[:, 21:22],
                    op0=OP.subtract, op1=OP.mult,
                )

            def transpose_tm_to_fm(h_tm, h_fm, name):
                """[128, 4, 768] bf16 token-major -> [128, 6, 512] fp8 f-major."""
                for db in range(NB_D):
                    trp = psB.tile([P, T_OWN], BF16, tag="psB", name=f"{name}_tr{db}")
                    for tb in range(NB_T):
                        nc.tensor.transpose(
                            trp[:, tb * P:(tb + 1) * P],
                            h_tm[:, tb, db * P:(db + 1) * P], idbf[:],
                        )
                    nc.vector.tensor_copy(h_fm[:, db, :], trp[:])

            # ---- initial stream ----
            x_t = sp.tile([P, NB_T, D], F32R, tag="stream", name="x0")
            nc.sync.dma_start(x_t[:], x_d[:])

            for l in range(L):
                # ---- weights for this layer ----
                wq = wp.tile([P, NB_D, D], FP8, tag="w8", bufs=3, name=f"wq{l}")
                nc.sync.dma_start(wq[:], wq_d[l].rearrange("p (k n) -> p k n", n=D))
                wv = wp.tile([P, NB_D, D], FP8, tag="w8", bufs=3, name=f"wv{l}")
                nc.sync.dma_start(wv[:], wv_d[l].rearrange("p (k n) -> p k n", n=D))
                bv_row = ap.tile([1, D], F32, tag="bvrow", bufs=2, name=f"bvr{l}")
                nc.sync.dma_start(bv_row[:], bvr_d[:, l, :])
                bo_row = ap.tile([1, D], BF16, tag="borow", bufs=2, name=f"bor{l}")
                nc.sync.dma_start(bo_row[:], bor_d[:, l, :])
                b2_row = ap.tile([1, D], BF16, tag="b2row", bufs=2, name=f"b2r{l}")
                nc.sync.dma_start(b2_row[:], b2r_d[:, l, :])

                # ---- LN1 + transpose to feature-major fp8 ----
                with nc.named_scope(f"L{l:02d}_a_ln1"):
                    h_tm = ap.tile([P, NB_T, D], BF16, tag="h_tm", bufs=1, name=f"h1tm{l}")
                    for tb in range(NB_T):
                        ln(h_tm[:, tb, :], x_t[:, tb, :])
                    h1_fm = ap.tile([P, NB_D, T_OWN], FP8, tag="h_fm", bufs=1,
                                    name=f"h1fm{l}")
                    transpose_tm_to_fm(h_tm, h1_fm, f"h1f{l}")
                    if DBG and l == 0:
                        nc.sync.dma_start(dbg["h1fm"][:], h1_fm[:])

                # ---- V token-major (+bias) -> send buffer (bf16) ----
                nc.enter_named_scope(f"L{l:02d}_b_v", False)
                bv_bc = ap.tile([P, D], F32, tag="bv_bc", bufs=1, name=f"bvbc{l}")
                nc.gpsimd.partition_broadcast(bv_bc[:], bv_row[:])
                v_send = ap.tile([P, NB_T, D], BF16, tag="vsend", bufs=1, name=f"vsend{l}")
                for tb in range(NB_T):
                    vp = psA.tile([P, 1024], F32, tag="psA", name=f"vps{l}_{tb}")
                    for n0, n1 in _regions():
                        for kp in range(NB_D // 2):
                            nc.tensor.matmul(
                                vp[:, n0:n1],
                                h1_fm[:, 2 * kp:2 * kp + 2, tb * P:(tb + 1) * P],
                                wv[:, 2 * kp:2 * kp + 2, n0:n1],
                                start=(kp == 0), stop=(kp == NB_D // 2 - 1),
                                perf_mode=PM.DoubleRow,
                            )
                    nc.vector.scalar_tensor_tensor(
                        out=v_send[:, tb, :], in0=vp[:, 0:D], scalar=1.0 / WS,
                        in1=bv_bc[:], op0=OP.mult, op1=OP.add,
                    )

                if DBG and l == 0:
                    nc.sync.dma_start(dbg["vsend"][:], v_send[:])
                nc.leave_named_scope(f"L{l:02d}_b_v", None, False)
                nc.enter_named_scope(f"L{l:02d}_c_ag", False)
                vsend_dr = dp.tile([NB_T, P, D], BF16, tag="vsend_d", name=f"vsdr{l}")
                for tb in range(NB_T):
                    nc.sync.dma_start(vsend_dr[tb], v_send[:, tb, :])
                vrecv_dr = dp.tile([NB_K, P, D], BF16, tag="vrecv_d", name=f"vrdr{l}")
                nc.gpsimd.collective_compute(
                    "AllGather", OP.bypass, replica_groups=REPLICA_GROUPS,
                    ins=[vsend_dr[:]], outs=[vrecv_dr[:]],
                )
                # own-half v_aug (cast bf16 -> fp8, strided into head blocks)
                va4 = v_aug
                for tb in range(NB_T):
                    nc.vector.tensor_copy(
                        va4[:, tb, :, 0:64],
                        v_send[:, tb, :].rearrange("p (h c) -> p h c", c=64),
                    )
                # own-half v_fm via PE transposes of v_send (bf16) -> fp8
                v_fm8 = ap.tile([P, NB_D, NB_K * P], FP8, tag="v_fm8", bufs=1,
                                name=f"vfm8{l}")
                for db in range(NB_D):
                    vto = psB.tile([P, T_OWN], BF16, tag="psB", name=f"vto{l}_{db}")
                    for tb in range(NB_T):
                        for hh in range(2):
                            nc.tensor.transpose(
                                vto[64 * hh:64 * hh + 64, tb * P:(tb + 1) * P],
                                v_send[:, tb, db * P + 64 * hh:db * P + 64 * hh + 64],
                                idbf[:],
                            )
                    nc.vector.tensor_copy(v_fm8[:, db, 0:T_OWN], vto[:])
                nc.leave_named_scope(f"L{l:02d}_c_ag", None, False)
                nc.enter_named_scope(f"L{l:02d}_d_q", False)
                # ---- Q (overlaps the AllGather) ----
                for m in range(NB_D):
                    qp = psB.tile([P, T_OWN], F32, tag="psB", name=f"qps{l}_{m}")
                    for kp in range(NB_D // 2):
                        nc.tensor.matmul(
                            qp[:], wq[:, 2 * kp:2 * kp + 2, m * P:(m + 1) * P],
                            h1_fm[:, 2 * kp:2 * kp + 2, :],
                            start=(kp == 0), stop=(kp == NB_D // 2 - 1),
                            perf_mode=PM.DoubleRow,
                        )
                    for hh in range(2):
                        r0 = 64 * hh
                        nc.vector.tensor_scalar(
                            out=q_par[hh][r0:r0 + 64, m, :], in0=qp[r0:r0 + 64, :],
                            scalar1=1.0 / WS, scalar2=bq_all[r0:r0 + 64, l, m:m + 1],
                            op0=OP.mult, op1=OP.add,
                        )

                # prefetch Wo while attention runs
                wo = wp.tile([P, NB_D, D], FP8, tag="w8", bufs=3, name=f"wo{l}")
                nc.sync.dma_start(wo[:], wo_d[l].rearrange("p (k n) -> p k n", n=D))

                nc.leave_named_scope(f"L{l:02d}_d_q", None, False)

                # ---- own-half scores for ALL dbs (overlaps AllGather) ----
                ex_own = {}
                nc.enter_named_scope(f"L{l:02d}_e_sown", False)
                for db in range(NB_D):
                    for pi, pair in enumerate(((0, 1), (2, 3))):
                        spv = [
                            psA.tile([P, 1024], F32, tag="psA",
                                     name=f"sco{l}_{db}_{pi}_{hh}")
                            for hh in range(2)
                        ]
                        for o in range(2):
                            kb = pair[o]
                            for hh in range(2):
                                nc.tensor.matmul(
                                    spv[hh][:, o * 512:(o + 1) * 512],
                                    v_fm8[:, db, kb * P:(kb + 1) * P],
                                    q_par[hh][:, db, :],
                                    start=True, stop=True,
                                    perf_mode=PM.DoublePixel,
                                )
                        ex = [
                            ap.tile([P, 2, 512], FP8, tag=f"exo{db}_{pi}_{hh}",
                                    bufs=1, name=f"exo{l}_{db}_{pi}_{hh}")
                            for hh in range(2)
                        ]
                        for hh in range(2):
                            nc.scalar.activation(
                                ex[hh].rearrange("p a b -> p (a b)"), spv[hh][:],
                                AF.Exp, scale=SCALE,
                            )
                        ex_own[(db, pi)] = ex
                if DBG and l == 0:
                    nc.sync.dma_start(dbg["qpar0"][:], q_par[0][:])
                    nc.sync.dma_start(dbg["qpar1"][:], q_par[1][:])
                    nc.sync.dma_start(dbg["ex00"][:], ex_own[(0, 0)][0][:])
                    nc.sync.dma_start(dbg["ex01"][:], ex_own[(0, 0)][1][:])
                nc.leave_named_scope(f"L{l:02d}_e_sown", None, False)

                # ---- peer half: recv, cast, transpose ----
                nc.enter_named_scope(f"L{l:02d}_f_vrecv", False)
                v_peer = ap.tile([P, NB_T, D], BF16, tag="vpeer", bufs=1, name=f"vpr{l}")
                peer_off = (1 - (nc.gpsimd.partition_id() % 2)) * NB_T
                nc.gpsimd.dma_start(v_peer[:], vrecv_dr[ds(peer_off, 4)])
                for tb in range(NB_T):
                    nc.vector.tensor_copy(
                        va4[:, NB_T + tb, :, 0:64],
                        v_peer[:, tb, :].rearrange("p (h c) -> p h c", c=64),
                    )
                for db in range(NB_D):
                    vtp = psB.tile([P, T_OWN], BF16, tag="psB", name=f"vtp{l}_{db}")
                    for tb in range(NB_T):
                        for hh in range(2):
                            nc.tensor.transpose(
                                vtp[64 * hh:64 * hh + 64, tb * P:(tb + 1) * P],
                                v_peer[:, tb, db * P + 64 * hh:db * P + 64 * hh + 64],
                                idbf[:],
                            )
                    nc.vector.tensor_copy(v_fm8[:, db, T_OWN:], vtp[:])
                if DBG and l == 0:
                    nc.sync.dma_start(dbg["vfm8"][:], v_fm8[:])
                    nc.sync.dma_start(dbg["vaug"][:], v_aug[:])
                nc.leave_named_scope(f"L{l:02d}_f_vrecv", None, False)

                # ---- peer scores + ctx per db (pipelined) ----
                nc.enter_named_scope(f"L{l:02d}_g_attn", False)
                ctx_n = ap.tile([P, NB_D, T_OWN], FP8, tag="ctx_n", bufs=1, name=f"ctxn{l}")
                for db in range(NB_D):
                    ex_all = [ex_own[(db, 0)], ex_own[(db, 1)]]
                    for pi, pair in enumerate(((4, 5), (6, 7))):
                        spv = [
                            psA.tile([P, 1024], F32, tag="psA",
                                     name=f"scp{l}_{db}_{pi}_{hh}")
                            for hh in range(2)
                        ]
                        for o in range(2):
                            kb = pair[o]
                            for hh in range(2):
                                nc.tensor.matmul(
                                    spv[hh][:, o * 512:(o + 1) * 512],
                                    v_fm8[:, db, kb * P:(kb + 1) * P],
                                    q_par[hh][:, db, :],
                                    start=True, stop=True,
                                    perf_mode=PM.DoublePixel,
                                )
                        ex = [
                            ap.tile([P, 2, 512], FP8, tag=f"exp{pi}_{hh}",
                                    bufs=2, name=f"exp{l}_{db}_{pi}_{hh}")
                            for hh in range(2)
                        ]
                        for hh in range(2):
                            nc.scalar.activation(
                                ex[hh].rearrange("p a b -> p (a b)"), spv[hh][:],
                                AF.Exp, scale=SCALE,
                            )
                        ex_all.append(ex)
                    ctxps = [
                        psB.tile([P, T_OWN], F32, tag="psB",
                                 name=f"ctxp{l}_{2 * db + hh}")
                        for hh in range(2)
                    ]
                    for hh in range(2):
                        h = 2 * db + hh
                        for j in range(4):
                            nc.tensor.matmul(
                                ctxps[hh][:],
                                v_aug[:, 2 * j:2 * j + 2, h, :],
                                ex_all[j][hh][:],
                                start=(j == 0), stop=(j == 3),
                                perf_mode=PM.DoubleRow,
                            )
                        r0 = 64 * hh
                        zsc = ap.tile([64, T_OWN], F32, tag="zsc", bufs=2,
                                      name=f"zs{l}_{2 * db + hh}")
                        nc.vector.tensor_scalar(
                            out=zsc[:], in0=ctxps[hh][64:128, :],
                            scalar1=1.0 / CS, scalar2=None, op0=OP.mult,
                        )
                        zinv = ap.tile([64, T_OWN], F32, tag="zinv", bufs=2,
                                       name=f"zi{l}_{2 * db + hh}")
                        nc.vector.reciprocal_approx_fast(out=zinv[:], in_=zsc[:])
                        nc.vector.tensor_tensor(
                            ctx_n[r0:r0 + 64, db, :], ctxps[hh][0:64, :],
                            zinv[:], OP.mult,
                        )
                if DBG and l == 0:
                    nc.sync.dma_start(dbg["ctxn"][:], ctx_n[:])
                nc.leave_named_scope(f"L{l:02d}_g_attn", None, False)

                # ---- Wo + residual (+ LN2 per block as it completes) ----
                nc.enter_named_scope(f"L{l:02d}_h_wo", False)
                skip = sp.tile([P, NB_T, D], F32R, tag="stream", name=f"skip{l}")
                h_tm2 = ap.tile([P, NB_T, D], BF16, tag="h_tm", bufs=1, name=f"h2tm{l}")
                for lb in range(NB_T):
                    wps = psA.tile([P, 1024], F32, tag="psA", name=f"wops{l}_{lb}")
                    for n0, n1 in _regions():
                        for kp in range(NB_D // 2):
                            nc.tensor.matmul(
                                wps[:, n0:n1],
                                ctx_n[:, 2 * kp:2 * kp + 2, lb * P:(lb + 1) * P],
                                wo[:, 2 * kp:2 * kp + 2, n0:n1],
                                start=(kp == 0), stop=False,
                                perf_mode=PM.DoubleRow,
                            )
                        nc.tensor.matmul(
                            wps[:, n0:n1], ones1[:], bo_row[:, n0:n1],
                            start=False, stop=True,
                        )
                    nc.vector.scalar_tensor_tensor(
                        out=skip[:, lb, :], in0=wps[:, 0:D], scalar=1.0 / (WS * CS),
                        in1=x_t[:, lb, :], op0=OP.mult, op1=OP.add,
                    )
                    ln(h_tm2[:, lb, :], skip[:, lb, :])
                if DBG and l == 0:
                    nc.sync.dma_start(dbg["skip"][:], skip[:])
                nc.leave_named_scope(f"L{l:02d}_h_wo", None, False)
                nc.enter_named_scope(f"L{l:02d}_i_ln2", False)
                h2_fm = ap.tile([P, NB_D, T_OWN], BF16, tag="h_fm2", bufs=1,
                                name=f"h2fm{l}")
                transpose_tm_to_fm(h_tm2, h2_fm, f"h2f{l}")
                nc.leave_named_scope(f"L{l:02d}_i_ln2", None, False)
                nc.enter_named_scope(f"L{l:02d}_j_ff", False)
                # ---- FFN (bf16): FF1 then FF2 (ck-outer, w2 streamed per half) ----
                g_all = ap.tile([P, NB_FF, T_OWN], BF16, tag="g", bufs=1, name=f"g{l}")
                for ck in range(4):
                    w1c = wp.tile([P, NB_D, D], BF16, tag="wbf", bufs=3, name=f"w1c{l}_{ck}")
                    nc.sync.dma_start(w1c[:], w1_d[l, ck].rearrange("p (k n) -> p k n", n=D))
                    for mm in range(NB_D):
                        fp = psB.tile([P, T_OWN], F32, tag="psB", name=f"f1ps{l}_{ck}_{mm}")
                        for kb in range(NB_D):
                            nc.tensor.matmul(
                                fp[:], w1c[:, kb, mm * P:(mm + 1) * P],
                                h2_fm[:, kb, :],
                                start=(kb == 0), stop=(kb == NB_D - 1),
                            )
                        nc.scalar.activation(
                            g_all[:, 6 * ck + mm, :], fp[:], AF.Gelu,
                            bias=b1_all[:, l, 6 * ck + mm:6 * ck + mm + 1],
                            scale=1.0,
                        )
                for half in range(2):
                    f2s = [
                        psA.tile([P, 1024], F32, tag="psA", name=f"f2ps{l}_{lb}")
                        for lb in (2 * half, 2 * half + 1)
                    ]
                    for ck in range(4):
                        w2c = wp.tile([P, NB_D, D], BF16, tag="wbf", bufs=3,
                                      name=f"w2c{l}_{half}_{ck}")
                        nc.sync.dma_start(
                            w2c[:], w2_d[l, ck].rearrange("p (k n) -> p k n", n=D))
                        for i, lb in enumerate((2 * half, 2 * half + 1)):
                            for n0, n1 in _regions():
                                for mm in range(NB_D):
                                    nc.tensor.matmul(
                                        f2s[i][:, n0:n1],
                                        g_all[:, 6 * ck + mm, lb * P:(lb + 1) * P],
                                        w2c[:, mm, n0:n1],
                                        start=(ck == 0 and mm == 0), stop=False,
                                    )
                    for i, lb in enumerate((2 * half, 2 * half + 1)):
                        for n0, n1 in _regions():
                            nc.tensor.matmul(
                                f2s[i][:, n0:n1], ones1[:], b2_row[:, n0:n1],
                                start=False, stop=True,
                            )
                        nc.vector.tensor_tensor(
                            skip[:, lb, :], skip[:, lb, :], f2s[i][:, 0:D], OP.add,
                        )
                nc.leave_named_scope(f"L{l:02d}_j_ff", None, False)
                x_t = skip

            nc.sync.dma_start(out_d[:], x_t[:])
    nc.compile()
    return nc


def _preprocess(inputs, n_layers):
    """Fold LN affine into projections; lay out weights for tile DMA."""
    f32 = np.float32
    L = n_layers
    Wq = np.asarray(inputs["Wq"], f32)[:L]
    Wv = np.asarray(inputs["Wv"], f32)[:L]
    Wo = np.asarray(inputs["Wo"], f32)[:L]
    W1 = np.asarray(inputs["W1"], f32)[:L]
    W2 = np.asarray(inputs["W2"], f32)[:L]
    g1 = np.asarray(inputs["ln1_g"], f32)[:L]
    b1ln = np.asarray(inputs["ln1_b"], f32)[:L]
    g2 = np.asarray(inputs["ln2_g"], f32)[:L]
    b2ln = np.asarray(inputs["ln2_b"], f32)[:L]
    bq = np.asarray(inputs["bq"], f32)[:L]
    bv = np.asarray(inputs["bv"], f32)[:L]
    bo = np.asarray(inputs["bo"], f32)[:L]
    b1 = np.asarray(inputs["b1"], f32)[:L]
    b2 = np.asarray(inputs["b2"], f32)[:L]

    Wq_eff = g1[:, :, None] * Wq
    bq_eff = bq + np.einsum("ld,ldo->lo", b1ln, Wq)
    Wv_eff = g1[:, :, None] * Wv
    bv_eff = bv + np.einsum("ld,ldo->lo", b1ln, Wv)
    W1_eff = g2[:, :, None] * W1
    b1_eff = b1 + np.einsum("ld,ldo->lo", b2ln, W1)

    def fm_weight(W):  # [L, D, D] -> [L, 128, 6*768] with [p, k, n]
        return np.ascontiguousarray(
            W.reshape(L, NB_D, P, D).transpose(0, 2, 1, 3).reshape(L, P, NB_D * D)
        )

    f8 = ml_dtypes.float8_e4m3
    bf = ml_dtypes.bfloat16

    def q8(W):
        return np.clip(W * WS, -240, 240).astype(f8)

    wq_h = q8(fm_weight(Wq_eff))
    wv_h = q8(fm_weight(Wv_eff))
    wo_h = q8(fm_weight(Wo))
    w1_h = np.ascontiguousarray(
        W1_eff.reshape(L, NB_D, P, 4, D).transpose(0, 3, 2, 1, 4).reshape(L, 4, P, NB_D * D)
    ).astype(bf)
    w2_h = np.ascontiguousarray(
        W2.reshape(L, 4, NB_D, P, D).transpose(0, 1, 3, 2, 4).reshape(L, 4, P, NB_D * D)
    ).astype(bf)
    bq_h = np.ascontiguousarray(bq_eff.reshape(L, NB_D, P).transpose(2, 0, 1))
    b1_h = np.ascontiguousarray(b1_eff.reshape(L, NB_FF, P).transpose(2, 0, 1))

    return {
        "wq": wq_h, "wv": wv_h, "wo": wo_h, "w1": w1_h, "w2": w2_h,
        "bq": bq_h, "b1": b1_h,
        "bv_row": np.ascontiguousarray(bv_eff[None]),
        "bo_row": np.ascontiguousarray(bo[None] * (WS * CS)).astype(bf),
        "b2_row": np.ascontiguousarray(b2[None]).astype(bf),
        "identbf": np.eye(P).astype(bf),
        "ones1": np.ones((1, P)).astype(bf),
    }


def kernel(**inputs) -> np.ndarray:
    n_layers = N_LAYERS
    key = ("nc", n_layers)
    if key not in _cached:
        _cached[key] = build(n_layers)
    nc = _cached[key]

    shared = _preprocess(inputs, n_layers)
    x = np.asarray(inputs["x"], np.float32)  # [4, 1024, 768]
    B, T, _ = x.shape

    in_maps = []
    for c in range(8):
        b, half = c // 2, c % 2
        x_own = x[b, half * T_OWN:(half + 1) * T_OWN]          # [512, 768]
        x_tile = np.ascontiguousarray(
            x_own.reshape(NB_T, P, D).transpose(1, 0, 2)        # [128, 4, 768]
        )
        in_maps.append({**shared, "x": x_tile})

    trace = bool(int(os.environ.get("KERNEL_TRACE", "0")))
    if trace:
        _register_ntff_hook()
    res = run_bass_kernel_spmd(nc, in_maps, core_ids=list(range(8)), trace=trace)
    global _last_results
    _last_results = res

    out = np.empty((B, T, D), dtype=np.float32)
    for c in range(8):
        b, half = c // 2, c % 2
        o = res.results[c]["out"]                               # [128, 4, 768]
        out[b, half * T_OWN:(half + 1) * T_OWN] = (
            o.transpose(1, 0, 2).reshape(T_OWN, D)
        )
    return out
